# revision 64
# baseline (speedup 1.0000x reference)
"""Trainium2 Bass kernel for nn_CausalAttentionKVCache (B=2, T=2048, D=1024, 16 heads).

Sharding: 8 cores = 2 batch-halves x 4 head-groups (4 heads each).
Two compiled SPMD programs (one per batch-half, phase constants differ mod 3),
dispatched concurrently on jax devices [0:4] and [4:8].

The module's reshape y.view(3,B,T,hs,nh) scrambles tokens: flat row
v = (c*B*T + b*T + t)//3 of y=[x@W+b] in column block j=(c*B*T+b*T+t)%3 holds
token t of tensor c (q/k/v). With a host-side column permutation of W
(W2[:, j*1024+h*64+d] = W[:, j*1024+d*16+h]) each head's 64 features are
contiguous, and each token-residue class (t mod 3) is a contiguous row-run.

All matmul operands are bf16 (f32 PSUM accumulate; cost model runs bf16 at
1 cycle/row at any moving size, so no f32r padding games are needed):
- Q^T/K^T = W^T @ x^T with features on partitions (Q descrambled on
  eviction; WQK/BQK are hp-major so the hp0 block DMAs first).
- V computed token-major ([128-token chunk, 4*65 feat cols]); the eviction
  adds a partition-replicated per-jj bias row whose 65th columns are 1.0,
  so the softmax denominator rides through PV and invalid (foreign-token)
  rows are excised from numerator and denominator by zeroing.
- S^T = K^T.T @ Q^T per (k-chunk, q-window of 512); exp on ScalarE
  (scale=1/8 fused, no max subtraction: scores ~ N(0,1)); causal staircase
  zeroed post-exp by gpsimd affine_select.
- PV transposed: stationary P [128 kpos, 128 qpos], moving V [128, 65]
  -> ctx accumulates in PSUM as [qpos, feats] (streams 65 rows per chunk
  instead of 512, and the output needs no PE transpose epilogue).  Both
  head accumulators share one PSUM bank (single start=True; bank-granular
  pending-zero covers the second stream's first write).
- Normalize = DVE reciprocal of the ones-column + per-partition scalar
  mul into a per-window staging tile, one out-DMA per window.

Scheduling: exp on the Act engine (~84us busy) and PE (~90us) are nearly
balanced, and the 3-deep S-psum pool caps how far exp can lag S.  The
emission therefore interleaves, per slot, the S-chunks of window k with
the PV batches of window k-2 plus projection fillers; projection/V psum
lives in the psctx pool so fillers never steal the S runway.  Window
order [1,2,3,0] per head-pair ramps S sizes [8,12,16,|8,12,16,4,4] and
keeps the 4-chunk windows (tiny exp) at the end; DMAs are issued in
need-order, one instruction per piece (HWDGE costs 625ns per DMA).
"""
import sys
import os

sys.path.insert(0, "/opt/trn_rl_repo")

import numpy as np

import concourse.bass as bass
import concourse.bacc as bacc
import concourse.mybir as mybir
import concourse.tile as tile

B, T, D, NH, HS = 2, 2048, 1024, 16, 64
NV = 684          # v-rows per (c, batch-half) slice
NVV = 772         # XTV slice width (guard col + window + pad)
GUARD = 1
NCHUNK = 6        # k/v chunks of 128 rows
QW = 512          # q window
F32 = mybir.dt.float32
BF16 = mybir.dt.bfloat16
F8 = mybir.dt.float8e4

_CACHE = {}


def _phase(B2):
    """Compile-time residue/offset constants for batch-half B2."""
    cst = {}
    for c in range(3):
        u0 = c * B * T + B2 * T
        vstart = u0 // 3
        rc_of_jj, r0_of_jj = {}, {}
        for rc in range(3):
            jj = (u0 + rc) % 3
            rc_of_jj[jj] = rc
            r0_of_jj[jj] = (u0 + rc - jj) // 3 - vstart
        cst[c] = dict(u0=u0, vstart=vstart, rc=rc_of_jj, r0=r0_of_jj)
    # rc-indexed views
    jk = {cst[1]["rc"][j]: j for j in range(3)}
    r0k = {cst[1]["rc"][j]: cst[1]["r0"][j] for j in range(3)}
    jv = {cst[2]["rc"][j]: j for j in range(3)}
    r0v = {cst[2]["rc"][j]: cst[2]["r0"][j] for j in range(3)}
    return cst, jk, r0k, jv, r0v


def _nrc(rc):
    return 683 if rc < 2 else 682


def _chunks(B2, q0):
    """Valid k-chunks (m, rc) for q-window [q0, q0+QW), with extents.

    a_e: matmul/exp start col, floored to 128 so PV's 128-wide stationary
         q-blocks never touch unwritten p_sb columns.
    a_o: first possibly-valid col (even); [a_e, a_o) is memset to 0.
    """
    _, jk, r0k, _, _ = _phase(B2)
    out = []
    for m in range(NCHUNK):
        for rc in range(3):
            t_min = rc + 3 * (128 * m - r0k[rc])
            if t_min >= q0 + QW:
                continue
            a = max(0, t_min - q0)
            out.append(dict(m=m, rc=rc, t_min=t_min, jjk=jk[rc],
                            a_e=a & ~127, a_o=a & ~1))
    return out


def _merge(primary, secondary):
    """Proportionally interleave two emit-closure lists (order-preserving)."""
    n, m = len(primary), len(secondary)
    if m == 0:
        return list(primary)
    if n == 0:
        return list(secondary)
    out = []
    si = 0
    for i, it in enumerate(primary):
        out.append(it)
        want = -(-((i + 1) * m) // n)   # ceil: secondaries lead slightly
        while si < want:
            out.append(secondary[si])
            si += 1
    out.extend(secondary[si:])
    return out


def _build_program(B2, repeat=1):
    cst, jk, r0k, jv, r0v = _phase(B2)
    nc = bacc.Bacc("TRN2", target_bir_lowering=False, debug=False, num_devices=4)

    xtq_d = nc.dram_tensor("XTQ", [D, NV], BF16, kind="ExternalInput")
    xtk_d = nc.dram_tensor("XTK", [D, NV], BF16, kind="ExternalInput")
    xtv_d = nc.dram_tensor("XTV", [D, NVV], BF16, kind="ExternalInput")
    wqk_d = nc.dram_tensor("WQK", [D, 768], BF16, kind="ExternalInput")
    wv_d = nc.dram_tensor("WV", [D, 780], BF16, kind="ExternalInput")
    bqk_d = nc.dram_tensor("BQK", [128, 6], F32, kind="ExternalInput")
    bvr_d = nc.dram_tensor("BVR", [128, 3, 260], BF16, kind="ExternalInput")
    out_d = nc.dram_tensor("OUT", [T, 256], F32, kind="ExternalOutput")

    with tile.TileContext(nc) as tc:
        with (
            tc.tile_pool(name="wpool", bufs=1) as wpool,
            tc.tile_pool(name="xpool", bufs=3) as xpool,
            tc.tile_pool(name="qkv", bufs=1) as qkvp,
            tc.tile_pool(name="ppool", bufs=38) as ppool,
            tc.tile_pool(name="opool", bufs=2) as opool,
            tc.tile_pool(name="rpool", bufs=4) as rpool,
            tc.tile_pool(name="pss", bufs=3, space="PSUM") as pss,
            tc.tile_pool(name="psctx", bufs=2, space="PSUM") as psctx,
        ):
            wqk = wpool.tile([128, 8, 768], BF16)
            wv = wpool.tile([128, 8, 780], BF16)
            bqk = wpool.tile([128, 6], F32)
            bvr = wpool.tile([128, 3, 260], BF16)

            for _rep in range(repeat):
                qt = qkvp.tile([128, 2, T], BF16, tag="qt")
                kt = qkvp.tile([128, 2, 3, 768], BF16, tag="kt")
                yvs = qkvp.tile([128, NCHUNK, 3, 260], BF16, tag="yvs")

                xtq = xpool.tile([128, 8, NVV], BF16, tag="xt", name="xq")
                xtk = xpool.tile([128, 8, NVV], BF16, tag="xt", name="xk")
                xtv = xpool.tile([128, 8, NVV], BF16, tag="xt", name="xv")
                # DMA in need-order, one instruction per piece (HWDGE's
                # 625ns fixed overhead per DMA makes fine splits expensive):
                # critical first pieces (wqk hp0 block, low-v x slices),
                # then the rest, then the V operands.
                RR = "(hl pr tw p) v -> p hl pr tw v"
                wqk_r = wqk_d.rearrange("(c p) f -> p c f", p=128)
                xtq_r = xtq_d.rearrange("(c p) v -> p c v", p=128)
                xtk_r = xtk_d.rearrange("(c p) v -> p c v", p=128)
                A = (slice(None),) * 4
                nc.sync.dma_start(wqk[:, 0:4, 0:128], wqk_r[:, 0:4, 0:128])
                nc.sync.dma_start(xtq[:, 0:4, 0:342], xtq_r[:, 0:4, 0:342])
                nc.sync.dma_start(wqk[:, 4:8, 0:128], wqk_r[:, 4:8, 0:128])
                nc.sync.dma_start(xtq[:, 4:8, 0:342], xtq_r[:, 4:8, 0:342])
                nc.sync.dma_start(bqk[:], bqk_d[:, :])
                nc.sync.dma_start(wqk[:, :, 128:384], wqk_r[:, :, 128:384])
                nc.sync.dma_start(xtk[:, 0:4, 0:384], xtk_r[:, 0:4, 0:384])
                nc.sync.dma_start(xtk[:, 4:8, 0:384], xtk_r[:, 4:8, 0:384])
                nc.sync.dma_start(xtq[:, :, 342:NV], xtq_r[:, :, 342:NV])
                nc.sync.dma_start(xtk[:, :, 384:NV], xtk_r[:, :, 384:NV])
                nc.sync.dma_start(wqk[:, :, 384:768], wqk_r[:, :, 384:768])
                nc.sync.dma_start(
                    xtv[:, :, :],
                    xtv_d.rearrange("(c p) v -> p c v", p=128)[:, :, :])
                nc.sync.dma_start(
                    wv[:, :, :],
                    wv_d.rearrange("(c p) f -> p c f", p=128)[:, :, :])
                nc.sync.dma_start(bvr[:], bvr_d[:, :, :])
                # zero the k tail cols (foreign-garbage keys are excised by
                # zeroed yvs rows, but unwritten SBUF would be NaN-poison)
                for hp in range(2):
                    for jz in range(3):
                        nc.gpsimd.memset(kt[:, hp, jz, NV:768], 0.0)

                # ---- projection emitters ----
                def emit_qk_part(si, jj, hp, vlo, vhi):
                    """One 128-feature block of Q (si=0) or K (si=1), v-range
                    [vlo, vhi).  8 accumulating matmuls + DVE bias eviction.
                    WQK/BQK are laid out hp-major: group g = hp*3 + jj."""
                    xt = (xtq, xtk)[si]
                    g = hp * 3 + jj
                    n = vhi - vlo
                    ps = psctx.tile([128, QW], F32, tag="ctx", name="psqk")
                    for ic in range(8):
                        nc.tensor.matmul(
                            ps[:, 0:n],
                            wqk[:, ic, g * 128:(g + 1) * 128],
                            xt[:, ic, vlo:vhi],
                            start=(ic == 0),
                            stop=(ic == 7),
                        )
                    if si == 0:   # Q: strided descramble eviction + bias
                        rc, r0 = cst[0]["rc"][jj], cst[0]["r0"][jj]
                        lo = max(vlo, r0)
                        hi = min(vhi, r0 + _nrc(rc))
                        if hi <= lo:
                            return
                        t0, t1 = rc + 3 * (lo - r0), rc + 3 * (hi - r0)
                        nc.vector.tensor_scalar_add(
                            qt[:, hp, t0:min(t1, T):3],
                            ps[:, lo - vlo:hi - vlo],
                            bqk[:, g:g + 1],
                        )
                    else:         # K: contiguous, v-indexed (keeps foreign
                        nc.vector.tensor_scalar_add(  # rows; benign, finite)
                            kt[:, hp, jj, vlo:vhi],
                            ps[:, 0:n],
                            bqk[:, g:g + 1],
                        )

                def emit_v_group(rc, m):
                    """One 128-token V chunk: [128, 4*65] with ones column.
                    Invalid rows (foreign tokens) are zeroed -> they drop out
                    of both the PV numerator and the denominator."""
                    jjv = jv[rc]
                    delta = r0v[rc] - r0k[rc]
                    r0 = r0k[rc]
                    lim = r0 + _nrc(rc)
                    mlo, plo = divmod(lim, 128)
                    if m > mlo or (m == mlo and plo == 0):
                        nc.gpsimd.memset(yvs[:, m, rc, :], 0.0)
                        return
                    ps = psctx.tile([128, QW], F32, tag="ctx", name="psv")
                    x0 = GUARD + 128 * m + delta
                    for ic in range(8):
                        nc.tensor.matmul(
                            ps[:, 0:260],
                            xtv[:, ic, x0:x0 + 128],
                            wv[:, ic, jjv * 260:(jjv + 1) * 260],
                            start=(ic == 0),
                            stop=(ic == 7),
                        )
                    # Eviction adds the per-jj replicated bias row (whose
                    # 65th columns are 1.0 -> denominator ones for free).
                    def evict(rows, mm, rr):
                        nc.vector.scalar_tensor_tensor(
                            yvs[0:rows, mm, rr, :], ps[0:rows, 0:260], 1.0,
                            bvr[0:rows, jjv, :],
                            op0=mybir.AluOpType.mult,
                            op1=mybir.AluOpType.add)
                    if m == mlo:
                        nc.gpsimd.memset(yvs[:, m, rc, :], 0.0)
                        evict(plo, m, rc)
                    elif m == 0 and r0 > 0:
                        evict(128, m, rc)
                        nc.gpsimd.memset(yvs[0:r0, m, rc, :], 0.0)
                    else:
                        evict(128, m, rc)

                # ---- attention emitters ----
                p_tiles = {}

                def emit_s_chunk(hp, q0, ci, ch):
                    """S matmuls + exp + causal select for one k-chunk."""
                    m, a_e, a_o = ch["m"], ch["a_e"], ch["a_o"]
                    s_ps = pss.tile([128, 2, QW], F32, tag="s", name="s_ps")
                    for hr in range(2):
                        pr = slice(hr * 64, hr * 64 + 64)
                        nc.tensor.matmul(
                            s_ps[:, hr, a_o:QW],
                            kt[pr, hp, ch["jjk"], 128 * m:128 * (m + 1)],
                            qt[pr, hp, q0 + a_o:q0 + QW],
                            start=True,
                            stop=True,
                            tile_position=(hr * 64, 0),
                        )
                    p_sb = ppool.tile([128, 2, QW], BF16, tag="p", name="p_sb")
                    if a_o > a_e:
                        nc.vector.memset(p_sb[:, :, a_e:a_o], 0.0)
                    nc.scalar.activation(
                        p_sb[:, :, a_o:QW],
                        s_ps[:, :, a_o:QW],
                        mybir.ActivationFunctionType.Exp,
                        scale=float(HS) ** -0.5,
                    )
                    ws, we = a_o, min(QW, ch["t_min"] + 382 - q0)
                    if ws < we:
                        nc.gpsimd.affine_select(
                            out=p_sb[:, :, ws:we],
                            in_=p_sb[:, :, ws:we],
                            pattern=[[0, 2], [1, we - ws]],
                            compare_op=mybir.AluOpType.is_ge,
                            fill=0.0,
                            base=q0 + ws - ch["t_min"],
                            channel_multiplier=-3,
                        )
                    p_tiles[(hp, q0, ci)] = p_sb

                def emit_pv_qb(hp, q0, qb, chunks, osb_box, split_dma):
                    """Accumulate ctx for one 128-query block over all its
                    valid chunks, normalize; the window's 4 blocks share one
                    staging tile and a single out-DMA."""
                    valid = [(ci, ch) for ci, ch in enumerate(chunks)
                             if ch["t_min"] < q0 + 128 * (qb + 1)]
                    # both hr accumulators share one psum bank: only the very
                    # first matmul carries start=True (bank-granular pending-
                    # zero covers the second stream's first write)
                    ctx = psctx.tile([128, QW], F32, tag="ctx", name="ctx")
                    for vi, (ci, ch) in enumerate(valid):
                        p_sb = p_tiles[(hp, q0, ci)]
                        for hr in range(2):
                            nc.tensor.matmul(
                                ctx[:, 65 * hr:65 * hr + 65],
                                p_sb[:, hr, 128 * qb:128 * (qb + 1)],
                                yvs[:, ch["m"], ch["rc"],
                                    (2 * hp + hr) * 65:(2 * hp + hr + 1) * 65],
                                start=(vi == 0 and hr == 0),
                                stop=(vi == len(valid) - 1 and hr == 1),
                                skip_group_check=True,
                            )
                    if qb == 0:
                        osb_box[0] = opool.tile([128, 4, 2, 64], F32,
                                                tag="o", name="osb")
                    osb = osb_box[0]
                    for hr in range(2):
                        rec = rpool.tile([128, 1], F32, tag="rec", name="rec")
                        nc.vector.reciprocal(rec[:], ctx[:, 65 * hr + 64:
                                                         65 * hr + 65])
                        nc.vector.tensor_scalar_mul(
                            osb[:, qb, hr, :],
                            ctx[:, 65 * hr:65 * hr + 64], rec[:])
                    if split_dma:
                        nc.sync.dma_start(
                            out_d[q0 + 128 * qb:q0 + 128 * (qb + 1),
                                  hp * 128:(hp + 1) * 128],
                            osb[:, qb, :, :],
                        )
                    elif qb == QW // 128 - 1:
                        nc.sync.dma_start(
                            out_d[q0:q0 + QW, hp * 128:(hp + 1) * 128]
                            .rearrange("(qb p) c -> p qb c", p=128),
                            osb[:],
                        )

                # ---- emission schedule ----
                def s_items(hp, q0):
                    chunks = _chunks(B2, q0)
                    return chunks, [
                        (lambda hp=hp, q0=q0, ci=ci, ch=ch:
                         emit_s_chunk(hp, q0, ci, ch))
                        for ci, ch in enumerate(chunks)
                    ]

                def pv_items(hp, q0, chunks, split_dma=False):
                    box = [None]
                    return [
                        (lambda hp=hp, q0=q0, qb=qb, chunks=chunks, box=box,
                                sd=split_dma:
                         emit_pv_qb(hp, q0, qb, chunks, box, sd))
                        for qb in range(QW // 128)
                    ]

                def qk_fill(si, jj, hp, vlo, vhi):
                    return lambda: emit_qk_part(si, jj, hp, vlo, vhi)

                def v_fill(rc, m):
                    return lambda: emit_v_group(rc, m)

                # lead-in: Q-hp0 mid (covers window-1 queries), K-hp0 m=0.
                # K parts are always emitted in the jj order S chunks
                # consume them (rc = 0,1,2 -> jj = jk[rc]).
                korder = [jk[rc] for rc in range(3)]
                # PE warm-up: dummy matmuls on the first-loaded wqk piece
                # start the 3us p-state ramp while the x slices stream in
                dps = pss.tile([128, 2, QW], F32, tag="s", name="dps")
                for dd in range(8):
                    nc.tensor.matmul(
                        dps[:, 0, 0:128],
                        wqk[:, 0, 0:128],
                        wqk[:, 1, 0:128],
                        start=(dd == 0),
                        stop=(dd == 7),
                    )
                for jj in range(3):
                    emit_qk_part(0, jj, 0, 170, 342)
                for jj in korder:
                    emit_qk_part(1, jj, 0, 0, 128)

                # Interleaved-hp window order, S sizes [8,12,8,16,12,16,4,4]:
                # every slot carries real exp work so the Act engine stays
                # saturated, fillers plug the remaining PE slack, and the two
                # 4-chunk windows land at the end (tiny PE tail).
                fillers = {
                    0: ([qk_fill(1, jj, 0, 128, 384) for jj in korder]
                        + [qk_fill(0, jj, 0, 342, NV) for jj in range(3)]
                        + [qk_fill(1, jj, 0, 384, NV) for jj in korder]),
                    1: ([qk_fill(0, jj, 1, 170, 342) for jj in range(3)]
                        + [qk_fill(1, jj, 1, 0, 128) for jj in korder]),
                    2: ([v_fill(rc, m) for m in (0, 1, 2) for rc in range(3)]
                        + [qk_fill(1, jj, 1, 128, 384) for jj in korder]
                        + [v_fill(rc, m) for m in (3, 4) for rc in range(3)]),
                    3: [qk_fill(0, jj, 1, 342, NV) for jj in range(3)],
                    4: ([qk_fill(1, jj, 1, 384, NV) for jj in korder]
                        + [v_fill(rc, 5) for rc in range(3)]),
                    5: [qk_fill(0, jj, 0, 0, 170) for jj in range(3)],
                    6: [qk_fill(0, jj, 1, 0, 170) for jj in range(3)],
                }

                wlist = [(0, 1), (0, 2), (0, 3), (1, 1),
                         (1, 2), (1, 3), (0, 0), (1, 0)]
                wlist = [(hp, w * QW) for hp, w in wlist]
                hist = []   # (hp, q0, chunks) per slot
                box7 = [None]
                for k, (hp, q0) in enumerate(wlist):
                    chunks, sitems = s_items(hp, q0)
                    sec = list(fillers.get(k, []))
                    if k >= 2:
                        ph, pq0, pch = hist[k - 2]
                        sec += pv_items(ph, pq0, pch)
                    if k == 7:
                        sec += pv_items(*hist[6], split_dma=True)
                        chunks7 = chunks
                        sec += [
                            (lambda qb=qb: emit_pv_qb(hp, q0, qb, chunks7,
                                                      box7, True))
                            for qb in range(QW // 128)
                        ]
                    for fn in _merge(sitems, sec):
                        fn()
                    hist.append((hp, q0, chunks))

    nc.compile()
    return nc


# ---------------------------------------------------------------------------
# host-side data prep
# ---------------------------------------------------------------------------

def _perm_cols():
    perm = np.empty(3 * D, dtype=np.int64)
    for j in range(3):
        for h in range(NH):
            for d in range(HS):
                perm[j * D + h * HS + d] = j * D + d * NH + h
    return perm


def _core_inputs(xT, W2, b2, B2, HG):
    """xT bf16 [D, B*T]; W2 bf16 [D, 3D]; b2 f32."""
    bf = xT.dtype
    cst, jk, r0k, jv, r0v = _phase(B2)

    def xt_slice(c, ncols, guard=0, src=None):
        src = xT if src is None else src
        vs = cst[c]["vstart"] - guard
        sl = np.zeros((src.shape[0], ncols), dtype=src.dtype)
        lo, hi = max(0, vs), min(B * T, vs + ncols)
        sl[:, lo - vs:hi - vs] = src[:, lo:hi]
        return sl

    # hp-major: group g = hp*3 + jj occupies cols [g*128, (g+1)*128)
    WQK = np.empty((D, 768), dtype=bf)
    BQKf = np.empty(768, dtype=np.float32)
    for hp in range(2):
        for jj in range(3):
            src = jj * D + HG * 256 + hp * 128
            g = hp * 3 + jj
            WQK[:, g * 128:(g + 1) * 128] = W2[:, src:src + 128]
            BQKf[g * 128:(g + 1) * 128] = b2[src:src + 128]
    BQK = BQKf.reshape(6, 128).T.copy()  # [128, 6]: col g, partition p

    WV = np.zeros((D, 780), dtype=bf)
    BVR = np.zeros((1, 3, 260), dtype=np.float32)
    for jj in range(3):
        for hl in range(4):
            src = jj * D + HG * 256 + hl * 64
            cb = (jj * 4 + hl) * 65
            WV[:, cb:cb + 64] = W2[:, src:src + 64]
            BVR[0, jj, hl * 65:hl * 65 + 64] = b2[src:src + 64]
            BVR[0, jj, hl * 65 + 64] = 1.0
    BVR = np.broadcast_to(BVR, (128, 3, 260)).copy()

    return {
        "XTQ": xt_slice(0, NV),
        "XTK": xt_slice(1, NV),
        "XTV": xt_slice(2, NVV, guard=GUARD),
        "WQK": WQK,
        "WV": WV,
        "BQK": np.ascontiguousarray(BQK),
        "BVR": BVR.astype(mybir.dt.np(BF16)),
    }


# ---------------------------------------------------------------------------
# concurrent two-program dispatch (4+4 cores)
# ---------------------------------------------------------------------------

def _sharded_fn(nc, dev_lo, dev_hi):
    import jax
    from jax.sharding import Mesh, PartitionSpec
    from jax.experimental.shard_map import shard_map
    from concourse import bass2jax
    from concourse.bass2jax import _bass_exec_p, install_neuronx_cc_hook

    install_neuronx_cc_hook()
    n_cores = dev_hi - dev_lo

    in_names, out_names, out_avals, zero_shapes = [], [], [], []
    partition_name = (
        nc.partition_id_tensor.name if nc.partition_id_tensor else None
    )
    for alloc in nc.m.functions[0].allocations:
        if not isinstance(alloc, mybir.MemoryLocationSet):
            continue
        name = alloc.memorylocations[0].name
        if alloc.kind == "ExternalInput":
            if name != partition_name:
                in_names.append(name)
        elif alloc.kind == "ExternalOutput":
            np_dt = mybir.dt.np(alloc.dtype)
            out_avals.append(
                jax.core.ShapedArray(tuple(alloc.tensor_shape), np_dt)
            )
            out_names.append(name)
            zero_shapes.append((tuple(alloc.tensor_shape), np_dt))
    n_params = len(in_names)
    all_in_names = list(in_names) + list(out_names)
    if partition_name is not None:
        all_in_names.append(partition_name)

    donate = tuple(range(n_params, n_params + len(out_names)))

    def _body(*args):
        operands = list(args)
        if partition_name is not None:
            operands.append(bass2jax.partition_id_tensor())
        outs = _bass_exec_p.bind(
            *operands,
            out_avals=tuple(out_avals),
            in_names=tuple(all_in_names),
            out_names=tuple(out_names),
            lowering_input_output_aliases=(),
            sim_require_finite=True,
            sim_require_nnan=True,
            nc=nc,
        )
        return tuple(outs)

    devices = jax.devices()[dev_lo:dev_hi]
    mesh = Mesh(np.asarray(devices), ("core",))
    in_specs = (PartitionSpec("core"),) * (n_params + len(out_names))
    out_specs = (PartitionSpec("core"),) * len(out_names)
    fn = jax.jit(
        shard_map(_body, mesh=mesh, in_specs=in_specs, out_specs=out_specs,
                  check_rep=False),
        donate_argnums=donate,
        keep_unused=True,
    )
    return fn, in_names, out_names, out_avals, zero_shapes, n_cores


def _concat_inputs(in_maps, in_names):
    return [
        np.concatenate([np.asarray(m[name]) for m in in_maps], axis=0)
        for name in in_names
    ]


def kernel(x, W_qkv, b_qkv):
    bf = mybir.dt.np(BF16)
    x = np.asarray(x, dtype=np.float32)
    W_qkv = np.asarray(W_qkv, dtype=np.float32)
    b_qkv = np.asarray(b_qkv, dtype=np.float32)

    if "progs" not in _CACHE:
        _CACHE["progs"] = {
            B2: _build_program(B2, repeat=int(os.environ.get("KREPEAT", "1")))
            for B2 in range(2)
        }
        _CACHE["fns"] = {
            0: _sharded_fn(_CACHE["progs"][0], 0, 4),
            1: _sharded_fn(_CACHE["progs"][1], 4, 8),
        }

    perm = _perm_cols()
    W2 = W_qkv[:, perm].astype(bf)
    b2 = b_qkv[perm]
    xT = np.ascontiguousarray(x.reshape(B * T, D).T).astype(bf)

    results = {}
    pending = []
    for B2 in range(2):
        fn, in_names, out_names, out_avals, zero_shapes, n_cores = _CACHE["fns"][B2]
        in_maps = [_core_inputs(xT, W2, b2, B2, HG) for HG in range(4)]
        concat_in = _concat_inputs(in_maps, in_names)
        concat_zeros = [
            np.zeros((n_cores * s[0], *s[1:]), d) for (s, d) in zero_shapes
        ]
        out_arrs = fn(*concat_in, *concat_zeros)  # async dispatch
        pending.append((B2, out_names, out_avals, n_cores, out_arrs))

    out_full = np.zeros((B, T, D), dtype=np.float32)
    for B2, out_names, out_avals, n_cores, out_arrs in pending:
        per_core = np.asarray(out_arrs[0]).reshape(n_cores, T, 256)
        for HG in range(4):
            out_full[B2, :, HG * 256:(HG + 1) * 256] = per_core[HG]
    return out_full
